# revision 1
# baseline (speedup 1.0000x reference)
"""Trainium2 Bass kernel for nn_ACMultiLayerdLSTM_73632919323053.

3-layer LSTM (U=1024) + dense(256) + LayerNorm + ReLU with auto-conditioning
over T=512 sequential steps, B=64, F=256.

Strategy: the recurrence is inherently sequential (no useful time
parallelism; per-step cross-core collectives cost >=18us so tensor
parallelism loses too). Every core runs the identical full-model program:
weights in bf16, W1 + part of W2 + Wd resident in SBUF (~19MB), the rest
(~24MB/step) streamed from HBM through a 6-slot ring at ~300GB/s,
fully unrolled over T with compile-time conditioning. PSUM double-wave
scheduling overlaps gate nonlinearities with the next wave's matmuls.
Output is taken from core 0.
"""
import numpy as np


import numpy as np
import ml_dtypes

import concourse.bass as bass
import concourse.mybir as mybir

AF = mybir.ActivationFunctionType
ALU = mybir.AluOpType
AX = mybir.AxisListType
BF16 = mybir.dt.bfloat16
F32 = mybir.dt.float32

B, F, U, L = 64, 256, 1024, 3
KT = [10, 16, 16]    # k-tiles per layer
LN_EPS = 1e-12
SEM_LIMIT = 50000
NRING = 6            # streamed half-chunk ring slots ([128, 2048] each)

# resident W2 k-tiles: the own-state rows consumed first (k = 8..)
R2 = 2
W2_RES = list(range(8, 8 + R2))

# k-row consumption order per layer (k-tile indices into the layer's K dim)
#   L0 rows: 0,1 = x tiles; 2..9 = h0 tiles 0..7  -> own-state h0 first
#   L1 rows: 0..7 = h0 (fresh); 8..15 = h1 (own)  -> h1 first
#   L2 rows: 0..7 = h1 (fresh); 8..15 = h2 (own)  -> h2 first
ROW_ORDER = {
    0: list(range(2, 10)) + [0, 1],
    1: list(range(8, 16)) + list(range(0, 8)),
    2: list(range(8, 16)) + list(range(0, 8)),
}

# streamed half-chunks in exact consumption order: (layer, ktile, half)
STREAM = []
for _h in (0, 1):
    STREAM += [(0, k, _h) for k in ROW_ORDER[0]]
for _h in (0, 1):
    STREAM += [(2, k, _h) for k in ROW_ORDER[2] if k not in W2_RES]
NS = len(STREAM)
STREAM_POS = {lkh: j for j, lkh in enumerate(STREAM)}


def gate_perm():
    """Column order [iA fA oA jA | iB fB oB jB]; A=units 0:512, B=512:1024.
    Original gate order (i, j, f, o)."""
    idx = np.arange(4 * U).reshape(4, U)
    g = [idx[0], idx[2], idx[3], idx[1]]  # i, f, o, j
    cols = []
    for h in range(2):
        for gi in range(4):
            cols.append(g[gi][h * 512:(h + 1) * 512])
    return np.concatenate(cols)


def prep_inmaps(inputs, T):
    perm = gate_perm()
    bf = ml_dtypes.bfloat16
    m = {}
    for l in range(L):
        W = np.asarray(inputs[f"W{l}"], np.float32)[:, perm]
        assert W.shape[0] == 128 * KT[l]
        m[f"W{l}"] = np.ascontiguousarray(W.reshape(KT[l], 128, 4 * U)).astype(bf)
    m["WD"] = np.ascontiguousarray(
        np.asarray(inputs["Wd"], np.float32).reshape(8, 128, F)).astype(bf)
    inp = np.asarray(inputs["input"], np.float32)
    xT = np.ascontiguousarray(inp.transpose(1, 2, 0))  # [Tfull, F, B]
    m["XT"] = np.ascontiguousarray(xT[:T].reshape(T, 2, 128, B).astype(bf))
    gamma = np.asarray(inputs["gamma"], np.float32)
    beta = np.asarray(inputs["beta"], np.float32)
    m["GB"] = np.ascontiguousarray(
        np.stack([gamma[:128], gamma[128:], beta[:128], beta[128:]], axis=1))
    m["IDENT"] = np.eye(64, dtype=np.float32)
    cond = np.asarray(inputs["conditioned_lst"]).astype(bool).copy()
    cond[0] = True
    assert not any(np.any(np.asarray(inputs[k])) for k in
                   ("b0", "b1", "b2", "bd")), "nonzero biases unsupported"
    return m, cond[:T]


def postprocess(outT):
    T = outT.shape[0]
    return np.ascontiguousarray(outT.reshape(T, F, B).transpose(2, 0, 1))


def build(T, cond, num_devices=8):
    nc = bass.Bass(num_devices=num_devices, debug=False)

    W0p = nc.declare_dram_parameter("W0", [KT[0], 128, 4 * U], BF16, isOutput=False)
    W1p = nc.declare_dram_parameter("W1", [KT[1], 128, 4 * U], BF16, isOutput=False)
    W2p = nc.declare_dram_parameter("W2", [KT[2], 128, 4 * U], BF16, isOutput=False)
    WDp = nc.declare_dram_parameter("WD", [8, 128, F], BF16, isOutput=False)
    XTp = nc.declare_dram_parameter("XT", [T, 2, 128, B], BF16, isOutput=False)
    GBp = nc.declare_dram_parameter("GB", [128, 4], F32, isOutput=False)
    IDp = nc.declare_dram_parameter("IDENT", [64, 64], F32, isOutput=False)
    OUTp = nc.declare_dram_parameter("out", [T, 2, 128, B], F32, isOutput=True)
    Wps = [W0p, W1p, W2p]

    from contextlib import ExitStack
    with ExitStack() as ctx:
        def sb(name, shape, dt):
            return ctx.enter_context(nc.sbuf_tensor(name, shape, dt))

        def ps(name, shape, dt):
            return ctx.enter_context(nc.psum_tensor(name, shape, dt))

        w1_sb = sb("w1_sb", [128, KT[1] * 4 * U], BF16)
        w2r_sb = sb("w2r_sb", [128, R2 * 4 * U], BF16)
        wd_sb = sb("wd_sb", [128, 8 * F], BF16)
        wring = sb("wring", [128, NRING * 2048], BF16)
        xin2 = sb("xin2", [128, 2 * 128], BF16)
        emitT_bf = sb("emitT_bf", [128, 128], BF16)
        emit_out = sb("emit_out", [128, 128], F32)
        hT_sb = sb("hT_sb", [128, L * 512], BF16)
        h_buf = sb("h_buf", [64, 1024], F32)
        gate_sb = sb("gate_sb", [64, 4 * 512], F32)
        tanhc_sb = sb("tanhc_sb", [64, 512], F32)
        c_sb = sb("c_sb", [64, L * U], F32)
        yc_sb = sb("yc_sb", [64, F], F32)
        sq_sb = sb("sq_sb", [64, F], F32)
        yhat_sb = sb("yhat_sb", [64, F], F32)
        ln_sb = sb("ln_sb", [64, 4], F32)
        rstd_sb = sb("rstd_sb", [64, 1], F32)
        gb_sb = sb("gb_sb", [128, 4], F32)
        id_sb = sb("id_sb", [64, 64], F32)
        pg = [ps(f"pg{i}", [128, 512], F32) for i in range(8)]
        block = ctx.enter_context(nc.Block())

        NR = KT[1] + R2 + 3
        SEMNAMES = [f"w{i}" for i in range(NRING)] + ["r", "in", "o", "p", "a", "d"]
        sem_handles = {n: [nc.alloc_semaphore(f"{n}0")] for n in SEMNAMES}

        def lhs_info(l, k):
            if l == 0:
                return ("x", k) if k < 2 else ("h", 0, k - 2)
            if l == 1:
                return ("h", 0, k) if k < 8 else ("h", 1, k - 8)
            return ("h", 1, k) if k < 8 else ("h", 2, k - 8)

        def resident(l, k):
            return l == 1 or (l == 2 and k in W2_RES)

        def gen(me, eng):
            base = {n: [0] for n in SEMNAMES}
            cnt = {n: 0 for n in SEMNAMES}

            def rotate(name):
                if cnt[name] - base[name][-1] > SEM_LIMIT:
                    base[name].append(cnt[name])
                    idx = len(base[name]) - 1
                    if idx >= len(sem_handles[name]):
                        sem_handles[name].append(
                            nc.alloc_semaphore(f"{name}{idx}"))

            last_wait = {}

            def W(name, abs_v):
                if abs_v is None or abs_v <= 0:
                    return
                bases = base[name]
                e = len(bases) - 1
                while e > 0 and bases[e] >= abs_v:
                    e -= 1
                rel = abs_v - bases[e]
                key = (name, e)
                if last_wait.get(key, -1) >= rel:
                    return
                last_wait[key] = rel
                eng.wait_ge(sem_handles[name][e], rel)

            def INC(inst, name, n):
                cnt[name] += n
                if inst is not None:
                    inst.then_inc(sem_handles[name][len(base[name]) - 1], n)
                return cnt[name]

            # ---------------- prologue ----------------
            if me == "sync":
                for k in range(KT[1]):
                    INC(eng.dma_start(out=w1_sb[:, k * 4096:(k + 1) * 4096],
                                      in_=W1p[k, :, :]), "r", 16)
                for i, k in enumerate(W2_RES):
                    INC(eng.dma_start(out=w2r_sb[:, i * 4096:(i + 1) * 4096],
                                      in_=W2p[k, :, :]), "r", 16)
                INC(eng.dma_start(
                    out=wd_sb[:, :].rearrange("p (k n) -> p k n", k=8),
                    in_=WDp.ap().rearrange("k p n -> p k n")), "r", 16)
                INC(eng.dma_start(out=gb_sb[:, :], in_=GBp[:, :]), "r", 16)
                INC(eng.dma_start(out=id_sb[:, :], in_=IDp[:, :]), "r", 16)
            else:
                for _ in range(NR):
                    INC(None, "r", 16)
            if me == "gpsimd":
                INC(eng.dma_start(
                    out=xin2[:, 0:128].rearrange("p (i b) -> p i b", i=2),
                    in_=XTp[0, :, :, :].rearrange("i p b -> p i b")), "in", 16)
            else:
                INC(None, "in", 16)
            R_ALL = cnt["r"]

            def chunk_dma(g):
                l_, k_, h_ = STREAM[g % NS]
                slot = g % NRING
                inst = None
                if me == "sync":
                    inst = eng.dma_start(
                        out=wring[:, slot * 2048:(slot + 1) * 2048],
                        in_=Wps[l_][k_, :, 2048 * h_:2048 * (h_ + 1)])
                INC(inst, f"w{slot}", 16)

            for g in range(min(NRING, NS * T)):
                chunk_dma(g)
            if me == "dve":
                INC(eng.memset(hT_sb[:, :], 0.0), "d", 1)
                INC(eng.memset(c_sb[:, :], 0.0), "d", 1)
                INC(eng.memset(emitT_bf[:, :], 0.0), "d", 1)
                INC(eng.memset(emit_out[:, :], 0.0), "d", 1)
            else:
                for _ in range(4):
                    INC(None, "d", 1)

            # planner state
            hT_guard = {(l, j): ("d", 1) for l in range(L) for j in range(8)}
            bank_guard = [None] * 8
            ring_release = {}
            xin_count = {0: 16}
            last_xin_read = {0: None, 1: None}
            emit_cast = None
            latest_h_done = None
            prev_tr_done = [None, None]
            h_done_half = [None, None]
            out_count = 0

            def dve_op(inst_fn, *waits):
                """DVE op with self-serialization + extra waits."""
                if me == "dve":
                    W("d", cnt["d"])
                    for wn, wv in waits:
                        W(wn, wv)
                    return INC(inst_fn(), "d", 1)
                return INC(None, "d", 1)

            for t in range(T):
                for n_ in SEMNAMES:
                    rotate(n_)

                # xin prefetch for t+1
                if t + 1 < T and cond[t + 1]:
                    par = (t + 1) % 2
                    if me == "gpsimd":
                        W("p", last_xin_read[par])
                        INC(eng.dma_start(
                            out=xin2[:, 128 * par:128 * (par + 1)]
                            .rearrange("p (i b) -> p i b", i=2),
                            in_=XTp[t + 1, :, :, :].rearrange("i p b -> p i b")),
                            "in", 16)
                    else:
                        INC(None, "in", 16)
                    xin_count[t + 1] = cnt["in"]

                import os
                NLAYERS = int(os.environ.get("KERN_DBG_LAYERS", L))
                for l in range(NLAYERS):
                    rows = ROW_ORDER[l]
                    nb_done = {}
                    for half in range(2):
                        for ri, k in enumerate(rows):
                            last_row = ri == len(rows) - 1
                            src = lhs_info(l, k)
                            if src[0] == "x":
                                xk = src[1]
                                if cond[t]:
                                    par = t % 2
                                    lhsT = xin2[:, 128 * par + 64 * xk:
                                                128 * par + 64 * xk + 64]
                                    lg = ("in", xin_count[t])
                                else:
                                    lhsT = emitT_bf[:, 64 * xk:64 * (xk + 1)]
                                    lg = emit_cast if emit_cast else ("d", 4)
                            else:
                                _, hl, hj = src
                                lhsT = hT_sb[:, 512 * hl + 64 * hj:
                                             512 * hl + 64 * (hj + 1)]
                                lg = hT_guard[(hl, hj)]
                            if me == "pe":
                                W(lg[0], lg[1])
                                if resident(l, k):
                                    W("r", R_ALL)
                                else:
                                    gi = t * NS + STREAM_POS[(l, k, half)]
                                    W(f"w{gi % NRING}",
                                      16 * (gi // NRING + 1))
                            for g4 in range(4):
                                bank = 4 * half + g4
                                if me == "pe":
                                    if ri == 0:
                                        bg = bank_guard[bank]
                                        if bg is not None:
                                            W(bg[0], bg[1])
                                    if resident(l, k):
                                        if l == 1:
                                            wsrc = w1_sb
                                            col = (k * 4096 + half * 2048 +
                                                   g4 * 512)
                                        else:
                                            wsrc = w2r_sb
                                            col = (W2_RES.index(k) * 4096 +
                                                   half * 2048 + g4 * 512)
                                        rhs = wsrc[:, col:col + 512]
                                    else:
                                        gi = t * NS + STREAM_POS[(l, k, half)]
                                        slot = gi % NRING
                                        rhs = wring[:, slot * 2048 + g4 * 512:
                                                    slot * 2048 + (g4 + 1) * 512]
                                    mm = eng.matmul(pg[bank][0:64, :], lhsT, rhs,
                                                    start=(ri == 0),
                                                    stop=last_row,
                                                    skip_group_check=True)
                                else:
                                    mm = None
                                if last_row:
                                    nb_done[4 * half + g4] = INC(mm, "p", 1)
                                elif g4 == 3:
                                    INC(mm, "p", 1)
                            if not resident(l, k):
                                gi = t * NS + STREAM_POS[(l, k, half)]
                                ring_release[gi] = cnt["p"]
                                gn = gi + NRING
                                if gn < NS * T:
                                    if me == "sync":
                                        W("p", ring_release[gi])
                                    chunk_dma(gn)
                            if src[0] == "x" and src[1] == 1 and cond[t]:
                                last_xin_read[t % 2] = cnt["p"]

                        # ---- nonlinearity + state for this wave ----
                        gslice = {gn_: gate_sb[:, 512 * gi_:512 * (gi_ + 1)]
                                  for gi_, gn_ in enumerate("ifoj")}
                        specs = [("i", AF.Sigmoid, 0.0), ("f", AF.Sigmoid, 1.0),
                                 ("o", AF.Sigmoid, 0.0), ("j", AF.Tanh, 0.0)]
                        for gidx, (gn_, fn, bias) in enumerate(specs):
                            bank = 4 * half + gidx
                            if me == "act":
                                W("p", nb_done[bank])
                                W("d", latest_h_done)
                                cp = eng.activation(gslice[gn_],
                                                    pg[bank][0:64, :],
                                                    fn, bias=bias)
                            else:
                                cp = None
                            ac = INC(cp, "a", 1)
                            bank_guard[bank] = ("a", ac)
                        a_gates = cnt["a"]
                        cs = c_sb[:, U * l + 512 * half:U * l + 512 * (half + 1)]
                        dve_op(lambda: eng.tensor_tensor(
                            gslice["j"], gslice["i"], gslice["j"], op=ALU.mult),
                            ("d", 4), ("a", a_gates))
                        dve_op(lambda: eng.tensor_tensor(
                            cs, cs, gslice["f"], op=ALU.mult))
                        c_done = dve_op(lambda: eng.tensor_tensor(
                            cs, cs, gslice["j"], op=ALU.add))
                        if me == "act":
                            W("d", c_done)
                            INC(eng.activation(tanhc_sb[:, :], cs, AF.Tanh),
                                "a", 1)
                        else:
                            INC(None, "a", 1)
                        tanhc_done = cnt["a"]
                        h_done = dve_op(lambda: eng.tensor_tensor(
                            h_buf[:, 512 * half:512 * (half + 1)],
                            gslice["o"], tanhc_sb[:, :], op=ALU.mult),
                            ("a", tanhc_done), ("p", prev_tr_done[half]))
                        latest_h_done = h_done
                        h_done_half[half] = h_done

                    # ---- transposes after BOTH waves (h(t-1) tiles must
                    # stay readable through wave B) ----
                    for half in range(2):
                        for tj in range(4):
                            bT = 4 * half + tj
                            if me == "pe":
                                W("d", h_done_half[half])
                                W("r", R_ALL)
                                bg = bank_guard[bT]
                                if bg is not None:
                                    W(bg[0], bg[1])
                                tr = eng.transpose(
                                    pg[bT][:, 0:64],
                                    h_buf[:, 512 * half + 128 * tj:
                                          512 * half + 128 * (tj + 1)],
                                    id_sb[:, :])
                            else:
                                tr = None
                            tr_done = INC(tr, "p", 1)
                            tile_idx = 4 * half + tj
                            if me == "act":
                                W("p", tr_done)
                                cp = eng.activation(
                                    hT_sb[:, 512 * l + 64 * tile_idx:
                                          512 * l + 64 * (tile_idx + 1)],
                                    pg[bT][:, 0:64], AF.Copy)
                            else:
                                cp = None
                            ac = INC(cp, "a", 1)
                            hT_guard[(l, tile_idx)] = ("a", ac)
                            bank_guard[bT] = ("a", ac)
                        prev_tr_done[half] = cnt["p"]

                # ---------------- dense + LN ----------------
                if NLAYERS < L:
                    # debug: skip dense/LN; fake the counters minimally
                    emit_done = dve_op(lambda: eng.tensor_scalar(
                        emit_out[:, :], emit_out[:, :], 0.0, None, op0=ALU.max))
                    if me == "act":
                        W("d", emit_done)
                        INC(eng.activation(emitT_bf[:, :], emit_out[:, :],
                                           AF.Copy), "a", 1)
                    else:
                        INC(None, "a", 1)
                    emit_cast = ("a", cnt["a"])
                    if me == "gpsimd":
                        W("d", emit_done)
                        INC(eng.dma_start(
                            out=OUTp[t, :, :, :].rearrange("i p b -> p i b"),
                            in_=emit_out[:, :].rearrange("p (i b) -> p i b", i=2)),
                            "o", 16)
                    else:
                        INC(None, "o", 16)
                    out_count = cnt["o"]
                    continue
                bY = 4   # wave-B bank (free after L2-waveB ACT reads)
                for k in range(8):
                    if me == "pe":
                        W(hT_guard[(2, k)][0], hT_guard[(2, k)][1])
                        if k == 0:
                            bg = bank_guard[bY]
                            if bg is not None:
                                W(bg[0], bg[1])
                        mm = eng.matmul(
                            pg[bY][0:64, 0:F],
                            hT_sb[:, 1024 + 64 * k:1024 + 64 * (k + 1)],
                            wd_sb[:, F * k:F * (k + 1)],
                            start=(k == 0), stop=(k == 7),
                            skip_group_check=True)
                    else:
                        mm = None
                    if k == 7:
                        y_done = INC(mm, "p", 1)
                dve_op(lambda: eng.tensor_reduce(
                    ln_sb[:, 0:1], pg[bY][0:64, 0:F], axis=AX.X, op=ALU.add),
                    ("p", y_done))
                musum_done = cnt["d"]
                if me == "act":
                    W("d", musum_done)
                    INC(eng.mul(ln_sb[:, 1:2], ln_sb[:, 0:1], 1.0 / F), "a", 1)
                else:
                    INC(None, "a", 1)
                mu_done = cnt["a"]
                yc_done = dve_op(lambda: eng.tensor_scalar(
                    yc_sb[:, :], pg[bY][0:64, 0:F], ln_sb[:, 1:2], None,
                    op0=ALU.subtract), ("a", mu_done))
                bank_guard[bY] = ("d", yc_done)
                if me == "act":
                    W("d", yc_done)
                    INC(eng.square(sq_sb[:, :], yc_sb[:, :]), "a", 1)
                else:
                    INC(None, "a", 1)
                sq_done = cnt["a"]
                dve_op(lambda: eng.tensor_reduce(
                    ln_sb[:, 2:3], sq_sb[:, :], axis=AX.X, op=ALU.add),
                    ("a", sq_done))
                varsum_done = cnt["d"]
                if me == "act":
                    W("d", varsum_done)
                    INC(eng.activation(ln_sb[:, 3:4], ln_sb[:, 2:3], AF.Sqrt,
                                       bias=0.0, scale=1.0 / F), "a", 1)
                else:
                    INC(None, "a", 1)
                sqv_done = cnt["a"]
                dve_op(lambda: eng.reciprocal(rstd_sb[:, :], ln_sb[:, 3:4]),
                       ("a", sqv_done))
                yhat_done = dve_op(lambda: eng.tensor_scalar(
                    yhat_sb[:, :], yc_sb[:, :], rstd_sb[:, :], None,
                    op0=ALU.mult))
                bY2 = 5
                for i in range(2):
                    if me == "pe":
                        W("d", yhat_done)
                        bg = bank_guard[bY2]
                        if bg is not None and i == 0:
                            W(bg[0], bg[1])
                        tr = eng.transpose(pg[bY2][:, 64 * i:64 * (i + 1)],
                                           yhat_sb[:, 128 * i:128 * (i + 1)],
                                           id_sb[:, :])
                    else:
                        tr = None
                    INC(tr, "p", 1)
                yT_done = cnt["p"]
                for i in range(2):
                    dve_op(lambda i=i: eng.tensor_scalar(
                        emit_out[:, 64 * i:64 * (i + 1)],
                        pg[bY2][:, 64 * i:64 * (i + 1)],
                        gb_sb[:, i:i + 1], gb_sb[:, 2 + i:3 + i],
                        op0=ALU.mult, op1=ALU.add),
                        ("p", yT_done), ("o", out_count))
                emit_done = dve_op(lambda: eng.tensor_scalar(
                    emit_out[:, :], emit_out[:, :], 0.0, None, op0=ALU.max))
                bank_guard[bY2] = ("d", emit_done)
                if me == "act":
                    W("d", emit_done)
                    INC(eng.activation(emitT_bf[:, :], emit_out[:, :],
                                       AF.Copy), "a", 1)
                else:
                    INC(None, "a", 1)
                emit_cast = ("a", cnt["a"])
                if me == "gpsimd":
                    W("d", emit_done)
                    INC(eng.dma_start(
                        out=OUTp[t, :, :, :].rearrange("i p b -> p i b"),
                        in_=emit_out[:, :].rearrange("p (i b) -> p i b", i=2)),
                        "o", 16)
                else:
                    INC(None, "o", 16)
                out_count = cnt["o"]

            if me == "gpsimd":
                W("o", out_count)

        @block.sync
        def _(eng):
            gen("sync", eng)

        @block.gpsimd
        def _(eng):
            gen("gpsimd", eng)

        @block.tensor
        def _(eng):
            gen("pe", eng)

        @block.scalar
        def _(eng):
            gen("act", eng)

        @block.vector
        def _(eng):
            gen("dve", eng)

    return nc


def kernel(**inputs):
    import concourse.bass as bass_mod  # noqa: F401  (env check)
    from concourse.bass_utils import run_bass_kernel_spmd

    T = int(np.asarray(inputs["input"]).shape[1])
    in_map, cond = prep_inmaps(inputs, T)
    nc = build(T, cond, num_devices=8)
    res = run_bass_kernel_spmd(
        nc, [in_map] * 8, core_ids=list(range(8)), trace=False)
    out = postprocess(np.asarray(res.results[0]["out"]))
    return out.astype(np.float32)

